# revision 14
# baseline (speedup 1.0000x reference)
"""Distributed attention kernel for 8 TRN2 NeuronCores (S^T-layout pipeline).

Problem: x[8192,1024] @ {W_q,W_k,W_v}[1024,128] -> softmax(QK^T/sqrt(128)) @ V.

Sharding: x row-sharded (1024 rows/core), weights replicated. Each core
computes K^T_loc/V_loc from its shard, AllGathers K^T (f32r) and V (bf16),
then attends its own 1024 Q rows against the full K/V.

Per-core pipeline (fully unrolled, Tile framework for sync):
  1. x^T via PE transposes; K^T projection (f32r) ASAP -> AllGather(K) issued
     early so V/Q projections overlap the collective.
  2. V projection bf16 -> AllGather(V), hidden under the stats phase.
  3. Stats (per 128-row q-tile): S = Q^T.T @ K^T in bf16; DVE row-max ->
     m_hat (only needs to be within ~80 of the true row max; bf16 err ~3).
  4. Main (per 512-col q-chunk, per 128-row kv-tile), software-pipelined
     emission so the PE never waits on the ACT exp:
       PSUM = K^T_tile.T @ Q^T_chunk (f32r)   [S^T: kv on partitions]
       PSUM += ones.T @ (-m_hat row)          [rank-1 max shift]
       A^T = exp(PSUM) via ACT -> bf16 SBUF
       O^T[dv,q] += V_tile(lhsT) @ A^T        [bf16]
       den[1,q]  += ones128.T @ A^T           [denominator]
     Stats for q-tiles 4-7 are interleaved into main chunk 0.
  5. O^T -> PE transpose -> scale by 1/den -> DMA out (woven into the next
     main chunk to keep the PE dense for HAM).

Numerics: logits have std ~1024 (randn inputs); softmax is near-one-hot so
the Q/K/S path needs |logit err| << 1: f32r (11-bit mantissa) gives ~0.15.
The shift value's accuracy is irrelevant (cancels in the normalization).
"""

import os
import sys

import numpy as np

os.environ.setdefault("MYCRO_LOCAL_CACHE", "1")

try:
    import concourse  # noqa: F401
except ImportError:  # pragma: no cover - path fallback for fresh dirs
    for _p in ("/opt/trn_rl_repo", "/root/.axon_site/_ro/trn_rl_repo"):
        if os.path.isdir(_p):
            sys.path.insert(0, _p)
    import concourse  # noqa: F401

import concourse.bass as bass
import concourse.mybir as mybir
import concourse.tile as tile
from concourse import bacc
from concourse.bass_utils import run_bass_kernel_spmd
from concourse.masks import make_identity

F32 = mybir.dt.float32
F32R = mybir.dt.float32r
BF16 = mybir.dt.bfloat16

N_CORES = 8
P = 128
NTOK = 8192
DIN = 1024
DQK = 128
DV = 128
NLOC = NTOK // N_CORES  # 1024 rows per core
TQ = NLOC // P  # 8 q tiles per core
TD = DIN // P  # 8 d_in tiles
NKV = NTOK // P  # 64 kv tiles
QC = 512  # q-chunk width for the main phase
NQC = NLOC // QC  # 2 q-chunks
SCH = 512  # stats chunk width (fp32 psum bank)
NSCH = NTOK // SCH  # 16 stats chunks per q-tile
SCALE = 1.0 / float(np.sqrt(DQK))
PIPE = 2  # software pipeline depth (kv tiles) between S^T matmul and AV


def build_nc():
    nc = bacc.Bacc(
        "TRN2",
        target_bir_lowering=False,
        debug=False,
        enable_asserts=False,
        num_devices=N_CORES,
    )

    x_d = nc.dram_tensor("x", [NLOC, DIN], F32, kind="ExternalInput").ap()
    wq_d = nc.dram_tensor("W_q", [DIN, DQK], F32, kind="ExternalInput").ap()
    wk_d = nc.dram_tensor("W_k", [DIN, DQK], F32, kind="ExternalInput").ap()
    wv_d = nc.dram_tensor("W_v", [DIN, DV], F32, kind="ExternalInput").ap()
    out_d = nc.dram_tensor("out", [NLOC, DV], F32, kind="ExternalOutput").ap()

    groups = [list(range(N_CORES))]

    with tile.TileContext(nc) as tc:
        with (
            tc.tile_pool(name="consts", bufs=1) as consts,
            tc.tile_pool(name="persist", bufs=1) as persist,
            tc.tile_pool(name="dram", bufs=1, space="DRAM") as dram,
        ):
            ident_f32 = consts.tile([P, P], F32)
            make_identity(nc, ident_f32)
            ones_f = consts.tile([1, P], F32)
            nc.vector.memset(ones_f, 1.0)
            ones_r = consts.tile([1, P], F32R)
            nc.vector.tensor_copy(out=ones_r, in_=ones_f)
            ones_col_bf = consts.tile([P, 1], BF16)
            nc.vector.memset(ones_col_bf, 1.0)

            # Persistent SBUF tensors.
            qT = persist.tile([P, NLOC], F32R)  # Q^T, pre-scaled, f32r
            qT_bf = persist.tile([P, NLOC], BF16)
            kT_full = persist.tile([P, NTOK], F32R)
            kT_bf = persist.tile([P, NTOK], BF16)
            vf = persist.tile([P, NKV, P], BF16)  # gathered V tiles
            kTl = persist.tile([P, NLOC], F32R)
            vl = persist.tile([P, TQ, P], BF16)
            negm_row = persist.tile([1, NLOC], F32)
            mx_all = persist.tile([P, TQ, NSCH], F32)

            # DRAM bounce buffers for the collectives.
            kv_bounce = dram.tile([P, NLOC], F32R)
            v_bounce = dram.tile([P, NLOC // 2], F32R)
            kv_gath = dram.tile(
                [P * N_CORES, NLOC], F32R, addr_space="Shared"
            )
            v_gath = dram.tile(
                [P * N_CORES, NLOC // 2], F32R, addr_space="Shared"
            )

            # ---------------- projections + collectives ----------------
            with (
                tc.tile_pool(name="proj_sb", bufs=1) as proj_sb,
                tc.tile_pool(name="ps_xt", bufs=2, space="PSUM") as ps_xt_pool,
                tc.tile_pool(name="ps_mm", bufs=2, space="PSUM") as ps_mm_pool,
                tc.tile_pool(name="ps_v", bufs=2, space="PSUM") as ps_v_pool,
            ):
                xa = proj_sb.tile([P, TQ, DIN], F32)
                xT_r = proj_sb.tile([P, TD, NLOC], F32R)
                xT_bf = proj_sb.tile([P, TD, NLOC], BF16)
                wq = proj_sb.tile([P, TD, DQK], F32)
                wk = proj_sb.tile([P, TD, DQK], F32)
                wv = proj_sb.tile([P, TD, DV], F32)
                wq_r = proj_sb.tile([P, TD, DQK], F32R)
                wk_r = proj_sb.tile([P, TD, DQK], F32R)
                wv_bf = proj_sb.tile([P, TD, DV], BF16)

                with nc.named_scope("load"):
                    for tj in range(TQ):
                        nc.sync.dma_start(
                            out=xa[:, tj, :], in_=x_d[tj * P : (tj + 1) * P, :]
                        )
                    nc.sync.dma_start(
                        out=wk, in_=wk_d.rearrange("(t p) d -> p t d", p=P)
                    )
                    nc.sync.dma_start(
                        out=wq, in_=wq_d.rearrange("(t p) d -> p t d", p=P)
                    )
                    nc.sync.dma_start(
                        out=wv, in_=wv_d.rearrange("(t p) d -> p t d", p=P)
                    )
                    nc.vector.tensor_copy(out=wk_r, in_=wk)
                    nc.vector.tensor_copy(out=wq_r, in_=wq)
                    nc.vector.tensor_copy(out=wv_bf, in_=wv)

                # x^T (PE transposes) and K^T projection per 512-token half,
                # so AllGather(K) can be issued as early as possible.
                for tg in range(2):
                    with nc.named_scope(f"xT_{tg}"):
                        for di in range(TD):
                            ps_xt = ps_xt_pool.tile([P, 4 * P], F32, tag="ps_xt")
                            for j in range(4):
                                tj = tg * 4 + j
                                nc.tensor.transpose(
                                    ps_xt[:, j * P : (j + 1) * P],
                                    xa[:, tj, di * P : (di + 1) * P],
                                    ident_f32,
                                )
                            sl = slice(tg * 4 * P, (tg + 1) * 4 * P)
                            nc.vector.tensor_copy(out=xT_r[:, di, sl], in_=ps_xt)
                    with nc.named_scope(f"kT_proj_{tg}"):
                        ps_k = ps_mm_pool.tile([P, 512], F32, tag="ps_mm")
                        for di in range(TD):
                            nc.tensor.matmul(
                                ps_k,
                                wk_r[:, di, :],
                                xT_r[:, di, tg * 512 : (tg + 1) * 512],
                                start=(di == 0),
                                stop=(di == TD - 1),
                            )
                        nc.vector.tensor_copy(
                            out=kTl[:, tg * 512 : (tg + 1) * 512], in_=ps_k
                        )
                        nc.sync.dma_start(
                            out=kv_bounce[:, tg * 512 : (tg + 1) * 512],
                            in_=kTl[:, tg * 512 : (tg + 1) * 512],
                        )

                with nc.named_scope("ag_k"):
                    nc.gpsimd.collective_compute(
                        "AllGather",
                        mybir.AluOpType.bypass,
                        replica_groups=groups,
                        ins=[kv_bounce.opt()],
                        outs=[kv_gath.opt()],
                    )
                with nc.named_scope("v_proj"):
                    for di in range(TD):
                        nc.vector.tensor_copy(
                            out=xT_bf[:, di, :], in_=xT_r[:, di, :].bitcast(F32)
                        )
                    for tj in range(TQ):
                        ps_v = ps_v_pool.tile([P, DV], F32, tag="ps_v")
                        for di in range(TD):
                            nc.tensor.matmul(
                                ps_v,
                                xT_bf[:, di, tj * P : (tj + 1) * P],
                                wv_bf[:, di, :],
                                start=(di == 0),
                                stop=(di == TD - 1),
                            )
                        nc.vector.tensor_copy(out=vl[:, tj, :], in_=ps_v)
                    nc.sync.dma_start(out=v_bounce.bitcast(BF16), in_=vl)

                with nc.named_scope("ag_v"):
                    nc.gpsimd.collective_compute(
                        "AllGather",
                        mybir.AluOpType.bypass,
                        replica_groups=groups,
                        ins=[v_bounce.opt()],
                        outs=[v_gath.opt()],
                    )

                with nc.named_scope("q_proj"):
                    for h in range(NLOC // 512):
                        ps_q = ps_mm_pool.tile([P, 512], F32, tag="ps_mm")
                        for di in range(TD):
                            nc.tensor.matmul(
                                ps_q,
                                wq_r[:, di, :],
                                xT_r[:, di, h * 512 : (h + 1) * 512],
                                start=(di == 0),
                                stop=(di == TD - 1),
                            )
                        nc.vector.tensor_scalar_mul(
                            qT[:, h * 512 : (h + 1) * 512], ps_q, SCALE
                        )
                    nc.vector.tensor_copy(out=qT_bf, in_=qT)

                with nc.named_scope("gather_k"):
                    for c in range(N_CORES):
                        nc.sync.dma_start(
                            out=kT_full[:, c * NLOC : (c + 1) * NLOC],
                            in_=kv_gath[c * P : (c + 1) * P, :],
                        )
                        nc.vector.tensor_copy(
                            out=kT_bf[:, c * NLOC : (c + 1) * NLOC],
                            in_=kT_full[:, c * NLOC : (c + 1) * NLOC].bitcast(F32),
                        )
                with nc.named_scope("gather_v"):
                    for c in range(N_CORES):
                        nc.sync.dma_start(
                            out=vf[:, c * TQ : (c + 1) * TQ, :],
                            in_=v_gath[c * P : (c + 1) * P, :]
                            .bitcast(BF16)
                            .rearrange("p (t d) -> p t d", d=P),
                        )

            # ---------------- attention ----------------
            with (
                tc.tile_pool(name="attn_sb", bufs=4) as attn_sb,
                tc.tile_pool(name="stat_sb", bufs=2) as stat_sb,
                tc.tile_pool(name="ps_stat", bufs=2, space="PSUM") as ps_stat_pool,
                tc.tile_pool(name="ps_st", bufs=4, space="PSUM") as ps_st_pool,
                tc.tile_pool(name="ps_od", bufs=1, space="PSUM") as ps_od_pool,
            ):

                def stats_unit(qt, ch):
                    """One stats chunk: bf16 matmul + DVE max."""
                    ps_stat = ps_stat_pool.tile([P, SCH], F32, tag="ps_stat")
                    nc.tensor.matmul(
                        ps_stat,
                        qT_bf[:, qt * P : (qt + 1) * P],
                        kT_bf[:, ch * SCH : (ch + 1) * SCH],
                        start=True,
                        stop=True,
                    )
                    nc.vector.reduce_max(
                        mx_all[:, qt, ch : ch + 1],
                        ps_stat,
                        axis=mybir.AxisListType.X,
                    )

                def stats_combine(qt):
                    """Combine chunk maxes -> -m_hat -> negm_row slice."""
                    m1 = stat_sb.tile([P, 1], F32, tag="m1")
                    negm = stat_sb.tile([P, 1], F32, tag="negm")
                    nc.vector.reduce_max(
                        m1, mx_all[:, qt, :], axis=mybir.AxisListType.X
                    )
                    nc.vector.tensor_scalar_mul(negm, m1, -1.0)
                    ps_nm = ps_stat_pool.tile([1, P], F32, tag="ps_stat")
                    nc.tensor.transpose(ps_nm, negm, ident_f32)
                    nc.vector.tensor_copy(
                        out=negm_row[0:1, qt * P : (qt + 1) * P], in_=ps_nm
                    )

                def st_mm(qc, kv, ps_st):
                    qs = qc * QC
                    nc.tensor.matmul(
                        ps_st,
                        kT_full[:, kv * P : (kv + 1) * P],
                        qT[:, qs : qs + QC],
                        start=True,
                        stop=True,
                    )

                def out_phase(qc, ps_o, ps_den, filler=None):
                    """Evacuate O^T + den for chunk qc: transpose, scale, DMA.

                    `filler()` emits PE work between steps to keep HAM warm.
                    """
                    qs = qc * QC
                    den_row = stat_sb.tile([1, QC], F32, tag="den_row")
                    rden_row = stat_sb.tile([1, QC], F32, tag="rden_row")
                    nc.vector.tensor_copy(out=den_row, in_=ps_den)
                    nc.vector.reciprocal(rden_row, den_row)
                    ps_rd = ps_stat_pool.tile([P, QC // P], F32, tag="ps_stat")
                    for j in range(QC // P):
                        nc.tensor.transpose(
                            ps_rd[:, j : j + 1],
                            rden_row[0:1, j * P : (j + 1) * P],
                            ones_f[0:1, 0:1],
                        )
                    rden_col = stat_sb.tile([P, QC // P], F32, tag="rden_col")
                    nc.vector.tensor_copy(out=rden_col, in_=ps_rd)

                    oT_sb = stat_sb.tile([P, QC], F32, tag="oT_sb")
                    nc.vector.tensor_copy(out=oT_sb, in_=ps_o)
                    o_nat = stat_sb.tile([P, QC // P, DV], F32, tag="o_nat")
                    ps_on = ps_st_pool.tile([P, QC], F32, tag="ps_st")
                    for j in range(QC // P):
                        nc.tensor.transpose(
                            ps_on[:, j * P : (j + 1) * P],
                            oT_sb[:, j * P : (j + 1) * P],
                            ident_f32,
                        )
                    for j in range(QC // P):
                        nc.vector.tensor_scalar_mul(
                            o_nat[:, j, :],
                            ps_on[:, j * P : (j + 1) * P],
                            rden_col[:, j : j + 1],
                        )
                    nc.sync.dma_start(
                        out=out_d[qs : qs + QC, :].rearrange(
                            "(t p) d -> p t d", p=P
                        ),
                        in_=o_nat,
                    )

                # stats for q-tiles of chunk 0
                with nc.named_scope("stats_a"):
                    for ch in range(NSCH):
                        for qt in range(TQ // 2):
                            stats_unit(qt, ch)
                    for qt in range(TQ // 2):
                        stats_combine(qt)

                # main chunk 0, with stats for q-tiles 4..7 interleaved and
                # software-pipelined so the PE never waits on ACT.
                stats_b = [
                    (qt, ch) for qt in range(TQ // 2, TQ) for ch in range(NSCH)
                ]
                for qc in range(NQC):
                    with nc.named_scope(f"main_{qc}"):
                        ps_o = ps_od_pool.tile([P, QC], F32, tag="ps_o", bufs=1)
                        ps_den = ps_od_pool.tile([1, QC], F32, tag="ps_den", bufs=1)
                        nb = attn_sb.tile([P, QC], F32, tag="nb", bufs=2)
                        nc.gpsimd.partition_broadcast(
                            nb, negm_row[0:1, qc * QC : (qc + 1) * QC]
                        )
                        st_tiles = {}
                        aT_tiles = {}
                        for kv in range(NKV + PIPE):
                            if kv < NKV:
                                ps_st = ps_st_pool.tile([P, QC], F32, tag="ps_st")
                                st_mm(qc, kv, ps_st)
                                st_tiles[kv] = ps_st
                                if qc == 0 and stats_b:
                                    stats_unit(*stats_b.pop(0))
                                nc.vector.tensor_tensor(
                                    ps_st, ps_st, nb, mybir.AluOpType.add
                                )
                                aT = attn_sb.tile([P, QC], BF16, tag="aT")
                                nc.scalar.activation(
                                    aT,
                                    st_tiles[kv],
                                    mybir.ActivationFunctionType.Exp,
                                )
                                aT_tiles[kv] = aT
                            k2 = kv - PIPE
                            if k2 >= 0:
                                nc.tensor.matmul(
                                    ps_o,
                                    vf[:, k2, :],
                                    aT_tiles[k2],
                                    start=(k2 == 0),
                                    stop=(k2 == NKV - 1),
                                )
                                nc.tensor.matmul(
                                    ps_den,
                                    ones_col_bf,
                                    aT_tiles[k2],
                                    start=(k2 == 0),
                                    stop=(k2 == NKV - 1),
                                )
                                del st_tiles[k2], aT_tiles[k2]
                        if qc == 0:
                            with nc.named_scope("stats_b_fin"):
                                while stats_b:
                                    stats_unit(*stats_b.pop(0))
                                for qt in range(TQ // 2, TQ):
                                    stats_combine(qt)
                    with nc.named_scope(f"out_{qc}"):
                        out_phase(qc, ps_o, ps_den)

    nc.compile()
    return nc


_NC_CACHE = None


def _get_nc():
    global _NC_CACHE
    if _NC_CACHE is None:
        _NC_CACHE = build_nc()
    return _NC_CACHE


def run(inputs, trace=False, **kw):
    """Run the SPMD kernel; returns BassKernelResults."""
    nc = _get_nc()
    x = np.asarray(inputs["x"], dtype=np.float32)
    wq = np.asarray(inputs["W_q"], dtype=np.float32)
    wk = np.asarray(inputs["W_k"], dtype=np.float32)
    wv = np.asarray(inputs["W_v"], dtype=np.float32)
    in_maps = [
        {
            "x": np.ascontiguousarray(x[c * NLOC : (c + 1) * NLOC]),
            "W_q": wq,
            "W_k": wk,
            "W_v": wv,
        }
        for c in range(N_CORES)
    ]
    return run_bass_kernel_spmd(
        nc, in_maps, core_ids=list(range(N_CORES)), trace=trace, **kw
    )


def kernel(**inputs):
    res = run(inputs, trace=False)
    return np.concatenate([res.results[c]["out"] for c in range(N_CORES)], axis=0)
